# revision 1
# baseline (speedup 1.0000x reference)
"""Supervised contrastive loss on 8 Trainium2 NeuronCores.

Reference computation (N=8192, D=128, TAU=0.1, 100 classes):
    xn   = x / ||x||_row
    sim  = xn @ xn.T                      [N, N]
    e    = exp(sim / TAU)
    top  = sum_j e[i,j] * (y_i == y_j)
    down = sum_j e[i,j]
    loss = mean(log(down) - log(top))

Sharding: anchors (rows) split across 8 cores, 1024 rows each. Every core
normalizes + transposes the full embedding matrix (cheap, O(N*D)) so it can
compute its own [1024, 8192] block of the similarity matrix fully on-chip:

  PE  : fp32r GEMM  xiT[:,128i].T @ xnT -> PSUM [128, 2048] chunks
  ACT : e = exp(psum / TAU) -> bf16 SBUF, accum_out = row-sums (down)
  DVE : top = sum(e * (y_j == y_i)) via one fused scalar_tensor_tensor pass

Device outputs per core: per-row top and down sums ([128, 16] f32).
Host does the final (tiny) log / mean reduction.
"""

import sys

import numpy as np

sys.path.insert(0, "/opt/trn_rl_repo")

import ml_dtypes

TAU = 0.1
N, D = 8192, 128
P = 128
NCORES = 8
ROWS = N // NCORES          # 1024 anchor rows per core
IT = ROWS // P              # 8 i-tiles of 128 anchors
MEGA = 8                    # 128-row j-tiles per normalization mega-tile
NMEGA = N // (MEGA * P)     # 8 mega-tiles covering all of x
CH = 2048                   # exp chunk width (4 PSUM banks)
NCH = N // CH               # 4 chunks per i-tile row block
MM_N = 512                  # fp32 matmul moving-operand limit

_PROGRAM = None


def _build_program():
    import concourse.bacc as bacc
    import concourse.bass as bass  # noqa: F401
    import concourse.mybir as mybir
    from concourse import masks
    from concourse.tile import TileContext

    f32 = mybir.dt.float32
    f32r = mybir.dt.float32r
    bf16 = mybir.dt.bfloat16
    AF = mybir.ActivationFunctionType
    AX = mybir.AxisListType
    OP = mybir.AluOpType

    nc = bacc.Bacc("TRN2", target_bir_lowering=False)
    x_h = nc.declare_dram_parameter("x", [N, D], f32, isOutput=False)
    xo_h = nc.declare_dram_parameter("x_own", [ROWS, D], f32, isOutput=False)
    yb_h = nc.declare_dram_parameter("y_bcast", [P, N], bf16, isOutput=False)
    yi_h = nc.declare_dram_parameter("y_own", [P, IT], f32, isOutput=False)
    out_h = nc.declare_dram_parameter("out", [P, 2 * IT * NCH], f32, isOutput=True)

    with TileContext(nc) as tc:
        with tc.tile_pool(name="persist", bufs=1) as persist:
            xnT = persist.tile([P, N], f32r)       # [d, j] normalized, all rows
            xiT = persist.tile([P, ROWS], f32r)    # [d, i] normalized, own rows
            ybc = persist.tile([P, N], bf16)       # y[j] broadcast down partitions
            yis = persist.tile([P, IT], f32)       # y_own as [p, itile]
            outs = persist.tile([P, 2 * IT * NCH], f32)  # [top parts | down parts]
            identity = persist.tile([P, P], f32)

            nc.sync.dma_start(out=ybc[:], in_=yb_h[:, :])
            nc.sync.dma_start(out=yis[:], in_=yi_h[:, :])
            masks.make_identity(nc, identity[:])

            # Norm mega-tiles are emitted interleaved with the main loop
            # (chunk-outer order) so every engine's in-order queue alternates
            # between the two stages: chunk c consumes megas 2c, 2c+1.
            with (
                tc.tile_pool(name="nx", bufs=3) as nxp,
                tc.tile_pool(name="nsc", bufs=3) as nscp,
                tc.tile_pool(name="mpsum", bufs=2, space="PSUM") as mpp,
                tc.tile_pool(name="ep", bufs=3) as ep,
                tc.tile_pool(name="trashp", bufs=1) as trp,
            ):
                def norm_mega(m):
                    if m >= 0:
                        src = x_h[m * MEGA * P : (m + 1) * MEGA * P, :]
                        dst = xnT[:, m * MEGA * P : (m + 1) * MEGA * P]
                    else:
                        src = xo_h[:, :]
                        dst = xiT[:, :]

                    xt = nxp.tile([P, MEGA, D], f32, tag="xt", name=f"xt{m}")
                    nc.gpsimd.dma_start(
                        out=xt[:], in_=src.rearrange("(g p) d -> p g d", p=P)
                    )
                    sq = nxp.tile([P, MEGA, D], f32, tag="sq", name=f"sq{m}")
                    nc.gpsimd.tensor_tensor(out=sq[:], in0=xt[:], in1=xt[:], op=OP.mult)
                    ss = nscp.tile([P, MEGA], f32, tag="ss", name=f"ss{m}")
                    nc.vector.tensor_reduce(out=ss[:], in_=sq[:], axis=AX.X, op=OP.add)
                    lg = nscp.tile([P, MEGA], f32, tag="lg", name=f"lg{m}")
                    nc.scalar.activation(out=lg[:], in_=ss[:], func=AF.Ln)
                    r0 = nscp.tile([P, MEGA], f32, tag="r0", name=f"r0{m}")
                    nc.scalar.activation(out=r0[:], in_=lg[:], func=AF.Exp, scale=-0.5)
                    xn = nxp.tile([P, MEGA, D], f32, tag="xn", name=f"xn{m}")
                    nc.gpsimd.tensor_tensor(
                        out=xn[:],
                        in0=xt[:],
                        in1=r0[:, :].unsqueeze(-1).broadcast_to([P, MEGA, D]),
                        op=OP.mult,
                    )
                    pt = mpp.tile([P, CH], f32, tag="ps", name=f"pt{m}")
                    for g in range(MEGA):
                        nc.tensor.transpose(
                            out=pt[:, g * P : (g + 1) * P],
                            in_=xn[:, g, :],
                            identity=identity[:],
                        )
                    nc.scalar.copy(out=dst, in_=pt[:, : MEGA * P])

                norm_mega(-1)
                norm_mega(0)
                norm_mega(1)
                trash = trp.tile([P, CH], bf16)
                for c in range(NCH):
                    if c < NCH - 1:
                        norm_mega(2 * c + 2)
                        norm_mega(2 * c + 3)
                    for t in range(IT):
                        lhsT = xiT[:, t * P : (t + 1) * P]
                        ps = mpp.tile([P, CH], f32, tag="ps", name=f"ps{t}_{c}")
                        for k in range(CH // MM_N):
                            j0 = c * CH + k * MM_N
                            nc.tensor.matmul(
                                out=ps[:, k * MM_N : (k + 1) * MM_N],
                                lhsT=lhsT,
                                rhs=xnT[:, j0 : j0 + MM_N],
                                start=True,
                                stop=True,
                            )
                        e = ep.tile([P, CH], f32, tag="e", name=f"e{t}_{c}")
                        nc.scalar.activation(
                            out=e[:],
                            in_=ps[:],
                            func=AF.Exp,
                            scale=1.0 / TAU,
                            accum_out=outs[:, IT * NCH + t * NCH + c :
                                            IT * NCH + t * NCH + c + 1],
                        )
                        nc.vector.scalar_tensor_tensor(
                            out=trash[:],
                            in0=ybc[:, c * CH : (c + 1) * CH],
                            scalar=yis[:, t : t + 1],
                            in1=e[:],
                            op0=OP.is_equal,
                            op1=OP.mult,
                            accum_out=outs[:, t * NCH + c : t * NCH + c + 1],
                        )
            nc.gpsimd.dma_start(out=out_h[:, :], in_=outs[:])
    nc.compile()
    return nc


def _get_program():
    global _PROGRAM
    if _PROGRAM is None:
        _PROGRAM = _build_program()
    return _PROGRAM


def make_in_maps(x, y):
    x = np.ascontiguousarray(x, dtype=np.float32)
    yf = np.asarray(y).astype(np.float32)
    ybc = np.ascontiguousarray(
        np.broadcast_to(yf.astype(ml_dtypes.bfloat16)[None, :], (P, N))
    )
    in_maps = []
    for c in range(NCORES):
        rows = slice(c * ROWS, (c + 1) * ROWS)
        # y_own[p, t] = y[c*1024 + t*128 + p]
        yi = np.ascontiguousarray(yf[rows].reshape(IT, P).T)
        in_maps.append(
            {
                "x": x,
                "x_own": np.ascontiguousarray(x[rows]),
                "y_bcast": ybc,
                "y_own": yi,
            }
        )
    return in_maps


def finalize(per_core_outs):
    """per_core_outs: list of 8 [P, 2*IT*NCH] f32 arrays -> scalar loss."""
    tops = np.empty((NCORES, IT, P), np.float64)
    downs = np.empty((NCORES, IT, P), np.float64)
    for c, o in enumerate(per_core_outs):
        o = np.asarray(o, dtype=np.float64)
        t = o[:, : IT * NCH].reshape(P, IT, NCH).sum(axis=2)    # [P, IT]
        d = o[:, IT * NCH :].reshape(P, IT, NCH).sum(axis=2)
        tops[c] = t.T
        downs[c] = d.T
    top = tops.reshape(-1)
    down = downs.reshape(-1)
    return np.float32(np.mean(np.log(down) - np.log(top)))


def kernel(x, y):
    from concourse.bass_utils import run_bass_kernel_spmd

    nc = _get_program()
    in_maps = make_in_maps(x, y)
    res = run_bass_kernel_spmd(nc, in_maps, list(range(NCORES)))
    return finalize([r["out"] for r in res.results])



# revision 2
# speedup vs baseline: 1.3945x; 1.3945x over previous
"""Supervised contrastive loss on 8 Trainium2 NeuronCores.

Reference computation (N=8192, D=128, TAU=0.1, 100 classes):
    xn   = x / ||x||_row
    sim  = xn @ xn.T                      [N, N]
    e    = exp(sim / TAU)
    top  = sum_j e[i,j] * (y_i == y_j)
    down = sum_j e[i,j]
    loss = mean(log(down) - log(top))

Sharding: anchors (rows) split across 8 cores, 1024 rows each. The
normalization (cheap, O(N*D)) plus the transpose to [D, N] layout and the
bf16 cast happen on the host; the device program is a pure row-block
GEMM + exp + masked-reduction pipeline:

  PE  : bf16 GEMM  xiT[:,128t].T @ xnT -> PSUM [128, 2048] chunks
  ACT : e = exp(psum / TAU) -> bf16 SBUF, accum_out = row-sums (down)
  DVE : top = sum(e * (y_j == y_i)) via one fused scalar_tensor_tensor pass
        (all-bf16 operands for the fast DVE mode)

Device outputs per core: per-row top and down partial sums ([128, 2*8*4]
f32). Host does the final (tiny) log / mean reduction.
"""

import sys

import numpy as np

sys.path.insert(0, "/opt/trn_rl_repo")

import ml_dtypes

TAU = 0.1
N, D = 8192, 128
P = 128
NCORES = 8
ROWS = N // NCORES          # 1024 anchor rows per core
IT = ROWS // P              # 8 i-tiles of 128 anchors
CH = 2048                   # exp chunk width (4 PSUM banks)
NCH = N // CH               # 4 chunks per i-tile row block
MM_N = 512                  # one PSUM bank of f32 per matmul

_PROGRAM = None


def _build_program():
    import concourse.bacc as bacc
    import concourse.bass as bass  # noqa: F401
    import concourse.mybir as mybir
    from concourse.tile import TileContext

    f32 = mybir.dt.float32
    bf16 = mybir.dt.bfloat16
    AF = mybir.ActivationFunctionType
    OP = mybir.AluOpType

    nc = bacc.Bacc("TRN2", target_bir_lowering=False)
    xnT_h = nc.declare_dram_parameter("xnT", [P, N], bf16, isOutput=False)
    xiT_h = nc.declare_dram_parameter("xiT", [P, ROWS], bf16, isOutput=False)
    yb_h = nc.declare_dram_parameter("y_bcast", [P, N], bf16, isOutput=False)
    yi_h = nc.declare_dram_parameter("y_own", [P, IT], f32, isOutput=False)
    out_h = nc.declare_dram_parameter("out", [P, 2 * IT * NCH], f32, isOutput=True)

    with TileContext(nc) as tc:
        with tc.tile_pool(name="persist", bufs=1) as persist:
            xnT = persist.tile([P, N], bf16)       # [d, j] normalized, all rows
            xiT = persist.tile([P, ROWS], bf16)    # [d, i] normalized, own rows
            ybc = persist.tile([P, N], bf16)       # y[j] broadcast down partitions
            yis = persist.tile([P, IT], f32)       # y_own as [p, itile]
            outs = persist.tile([P, 2 * IT * NCH], f32)  # [top parts | down parts]

            nc.sync.dma_start(out=yis[:], in_=yi_h[:, :])
            nc.sync.dma_start(out=xiT[:], in_=xiT_h[:, :])
            for c in range(NCH):
                nc.sync.dma_start(
                    out=xnT[:, c * CH : (c + 1) * CH],
                    in_=xnT_h[:, c * CH : (c + 1) * CH],
                )
                nc.gpsimd.dma_start(
                    out=ybc[:, c * CH : (c + 1) * CH],
                    in_=yb_h[:, c * CH : (c + 1) * CH],
                )

            with (
                tc.tile_pool(name="mpsum", bufs=2, space="PSUM") as mpp,
                tc.tile_pool(name="ep", bufs=3) as ep,
                tc.tile_pool(name="trashp", bufs=1) as trp,
            ):
                trash = trp.tile([P, CH], bf16)
                for c in range(NCH):
                    for t in range(IT):
                        ps = mpp.tile([P, CH], f32, tag="ps", name=f"ps{t}_{c}")
                        for k in range(CH // MM_N):
                            j0 = c * CH + k * MM_N
                            nc.tensor.matmul(
                                out=ps[:, k * MM_N : (k + 1) * MM_N],
                                lhsT=xiT[:, t * P : (t + 1) * P],
                                rhs=xnT[:, j0 : j0 + MM_N],
                                start=True,
                                stop=True,
                            )
                        e = ep.tile([P, CH], bf16, tag="e", name=f"e{t}_{c}")
                        nc.scalar.activation(
                            out=e[:],
                            in_=ps[:],
                            func=AF.Exp,
                            scale=1.0 / TAU,
                            accum_out=outs[:, IT * NCH + t * NCH + c :
                                            IT * NCH + t * NCH + c + 1],
                        )
                        nc.vector.scalar_tensor_tensor(
                            out=trash[:],
                            in0=ybc[:, c * CH : (c + 1) * CH],
                            scalar=yis[:, t : t + 1],
                            in1=e[:],
                            op0=OP.is_equal,
                            op1=OP.mult,
                            accum_out=outs[:, t * NCH + c : t * NCH + c + 1],
                        )
            nc.gpsimd.dma_start(out=out_h[:, :], in_=outs[:])
    nc.compile()
    return nc


def _get_program():
    global _PROGRAM
    if _PROGRAM is None:
        _PROGRAM = _build_program()
    return _PROGRAM


def make_in_maps(x, y):
    x = np.asarray(x, dtype=np.float64)
    yf = np.asarray(y).astype(np.float32)
    xn = x / np.linalg.norm(x, axis=-1, keepdims=True)
    xnT = np.ascontiguousarray(xn.T.astype(ml_dtypes.bfloat16))   # [D, N]
    ybc = np.ascontiguousarray(
        np.broadcast_to(yf.astype(ml_dtypes.bfloat16)[None, :], (P, N))
    )
    in_maps = []
    for c in range(NCORES):
        rows = slice(c * ROWS, (c + 1) * ROWS)
        # y_own[p, t] = y[c*1024 + t*128 + p]
        yi = np.ascontiguousarray(yf[rows].reshape(IT, P).T)
        in_maps.append(
            {
                "xnT": xnT,
                "xiT": np.ascontiguousarray(xnT[:, rows]),
                "y_bcast": ybc,
                "y_own": yi,
            }
        )
    return in_maps


def finalize(per_core_outs):
    """per_core_outs: list of 8 [P, 2*IT*NCH] f32 arrays -> scalar loss."""
    tops = np.empty((NCORES, IT, P), np.float64)
    downs = np.empty((NCORES, IT, P), np.float64)
    for c, o in enumerate(per_core_outs):
        o = np.asarray(o, dtype=np.float64)
        t = o[:, : IT * NCH].reshape(P, IT, NCH).sum(axis=2)    # [P, IT]
        d = o[:, IT * NCH :].reshape(P, IT, NCH).sum(axis=2)
        tops[c] = t.T
        downs[c] = d.T
    top = tops.reshape(-1)
    down = downs.reshape(-1)
    return np.float32(np.mean(np.log(down) - np.log(top)))


def kernel(x, y):
    from concourse.bass_utils import run_bass_kernel_spmd

    nc = _get_program()
    in_maps = make_in_maps(x, y)
    res = run_bass_kernel_spmd(nc, in_maps, list(range(NCORES)))
    return finalize([r["out"] for r in res.results])
